# revision 4
# baseline (speedup 1.0000x reference)
"""LoKr linear forward on 8 TRN2 NeuronCores — hybrid bf16/fp8 matmul.

out = x @ (W0 + (alpha/lora_dim) * kron(w1, w2_a @ w2_b)).T + b

Same data-parallel token sharding as the baseline, but the K=4096
contraction is split: the first KB dims run in bf16 (exact to ~1e-3),
the remaining KF dims run in fp8-e4m3 DoubleRow mode (2x TensorE rate,
256-deep contraction per instruction). Measured rel err ~1.8e-2 < 2e-2.

Weights (both parts) and bias are pre-scaled by 32 on host so the fp8
weight values clear the e4m3 subnormal range; the kernel output is
32*(x@W+b) and the host multiplies by 1/32 (exact) after gather.
"""
import sys, types

sys.path.insert(0, '/opt/trn_rl_repo')

import numpy as np
import ml_dtypes
import concourse.bass as bass
import concourse.mybir as mybir
import concourse.tile as tile
import concourse.bass_utils as bass_utils

ALPHA = 1.0
LORA_DIM = 4
MULTIPLIER = 1.0

N_CORES = 8
B, S, IN, OUT = 4, 4096, 4096, 4096
T_CORE = B * S // N_CORES          # 2048 tokens per core
T_HALF = T_CORE // 2               # 1024
KT = 128                           # bf16 contraction tile
TT = 128                           # token tile (psum partitions)
OT = 512                           # out-feature tile (psum free dim)
KB = 2560                          # bf16 contraction span
KF = IN - KB                       # fp8 contraction span (1280)
NKB = KB // KT                     # 22 bf16 k-tiles
NKP = KF // 256                    # 5 fp8 DoubleRow k-pair-tiles
KFB = KF // 128                    # 10 fp8 k-blocks
NO = OUT // OT                     # 8
NT = T_HALF // TT                  # 8
WSCALE = 32.0


def _split_multi_waits(nc):
    """This walrus build encodes at most ONE semaphore wait per ISA
    instruction; hoist extra waits onto single-wait NOPs inserted before."""
    ctr = 0
    for f in nc.m.functions:
        for blk in f.blocks:
            out = []
            changed = False
            for i in blk.instructions:
                si = i.sync_info
                if si is not None and si.on_wait and len(si.on_wait) > 1:
                    waits = list(si.on_wait)
                    for w in waits[:-1]:
                        ctr += 1
                        out.append(mybir.InstNoOp(
                            name=f"I-wsplit-{ctr}",
                            engine=i.engine, ins=[], outs=[],
                            sync_info=mybir.SyncInfo(on_wait=[w], on_update=[]),
                        ))
                    i.sync_info = mybir.SyncInfo(
                        on_wait=[waits[-1]], on_update=list(si.on_update))
                    changed = True
                out.append(i)
            if changed:
                blk.instructions = out


def build_nc():
    nc = bass.Bass(trn_type="TRN2")
    bf16 = mybir.dt.bfloat16
    f8 = mybir.dt.float8e4
    f32 = mybir.dt.float32
    DR = mybir.MatmulPerfMode.DoubleRow
    xT = nc.dram_tensor("xT", [KB, T_CORE], bf16, kind="ExternalInput")
    # x8 packed per half: [th][128 p][KFB][1024 t], value = x.T[KB+kb*128+p, t]
    x8d = nc.dram_tensor("x8", [2, 128, KFB, T_HALF], f8, kind="ExternalInput")
    wT = nc.dram_tensor("wT", [KB, OUT], bf16, kind="ExternalInput")
    # w8 packed per o-block: [ob][128 p][KFB][512 o]
    w8d = nc.dram_tensor("w8", [NO, 128, KFB, OT], f8, kind="ExternalInput")
    bias_d = nc.dram_tensor("bias", [128, OUT], f32, kind="ExternalInput")
    out = nc.dram_tensor("out", [T_CORE, OUT], f32, kind="ExternalOutput")

    with tile.TileContext(nc) as tc:
        with (
            tc.tile_pool(name="const", bufs=1) as constp,
            tc.tile_pool(name="warm", bufs=1) as warmp,
            tc.tile_pool(name="xp", bufs=NKB + 4) as xp,
            tc.tile_pool(name="xp8", bufs=2) as xp8,
            tc.tile_pool(name="w0p", bufs=1) as w0p,       # o=0 W, resident
            tc.tile_pool(name="wp", bufs=2 * NKB) as wp,
            tc.tile_pool(name="wp8", bufs=2) as wp8,
            tc.tile_pool(name="op", bufs=8) as op,
            tc.tile_pool(name="ps", bufs=8, space="PSUM") as pp,
        ):
            # PE warm-up: keep TensorE busy while the first DMAs land so the
            # HAM clock-gate opens before real matmuls start. (wz is read
            # uninitialized on purpose; the result is never consumed.)
            wz = warmp.tile([KT, OT], bf16)
            nc.sync.dma_start(wz[:], wT[0:KT, 0:OT])
            wps = pp.tile([TT, OT], f32, tag="ps")
            for _ in range(96):
                nc.tensor.matmul(wps[:, :TT], wz[:, :TT], wz[:, :TT],
                                 start=True, stop=True)

            bias = constp.tile([128, OUT], f32)

            # o=0 weights are th-invariant: load once, keep resident.
            # NOTE: this pre-compute DMA wave (before the first real matmul)
            # is deliberate: the ~15-20us PE-idle window after the warm-up
            # burst is what lets the HAM clock-gate step the PE clock to
            # 2.4 GHz. A "better-overlapped" schedule that keeps the PE
            # continuously busy from t=0 leaves the clock stuck ~20% lower
            # for the whole kernel (measured 890us vs 763us).
            w0ts = []
            for k in range(NKB):
                wt = w0p.tile([KT, OT], bf16, name=f"w0_{k}")
                nc.sync.dma_start(wt[:], wT[k*KT:(k+1)*KT, 0:OT])
                w0ts.append(wt)
            w08 = w0p.tile([128, KFB, OT], f8, name="w0_f8")
            nc.sync.dma_start(w08[:], w8d[0])

            for th in range(2):
                t0 = th * T_HALF
                # fp8 x half-shard first (one big DMA), then bf16 x tiles.
                x8t = xp8.tile([128, KFB, T_HALF], f8, tag="x8")
                nc.sync.dma_start(x8t[:], x8d[th])
                xts = [xp.tile([KT, T_HALF], bf16, tag="x",
                               name=f"x_{th}_{k}")
                       for k in range(NKB)]
                for k in range(1 if th == 0 else 0, NKB):
                    nc.sync.dma_start(
                        xts[k][:], xT[k*KT:(k+1)*KT, t0:t0+T_HALF])
                if th == 0:
                    nc.sync.dma_start(bias[:], bias_d[:])
                    # x_0 is deliberately the LAST DMA of the prefetch wave:
                    # the first real matmul then waits out the full wave
                    # (~35us PE-idle after the warm-up burst), which the HAM
                    # clock-gate needs to step the PE to 2.4 GHz regardless
                    # of the clock state a previous kernel left behind.
                    # Without this the same binary nondeterministically runs
                    # ~20% slower end to end (907us vs 761us measured).
                    nc.sync.dma_start(xts[0][:], xT[0:KT, t0:t0+T_HALF])
                for o in range(NO):
                    if o == 0:
                        # k-outer / t-inner across all 8 PSUM banks: the PE
                        # consumes each freshly-DMA'd tile for all 8 token
                        # tiles at once, tracking the DMA wavefront.
                        pss = [pp.tile([TT, OT], f32, tag="ps",
                                        name=f"pss_{th}_{i}")
                               for i in range(NT)]
                        for k in range(NKB):
                            for tt in range(NT):
                                nc.tensor.matmul(
                                    pss[tt][:],
                                    xts[k][:, tt*TT:(tt+1)*TT], w0ts[k][:],
                                    start=(k == 0), stop=False)
                        for kp in range(NKP):
                            for tt in range(NT):
                                nc.tensor.matmul(
                                    pss[tt][:],
                                    x8t[:, 2*kp:2*kp+2, tt*TT:(tt+1)*TT],
                                    w08[:, 2*kp:2*kp+2, :],
                                    start=False, stop=(kp == NKP - 1),
                                    perf_mode=DR)
                                if kp == NKP - 1:
                                    ot = op.tile([TT, OT], f32, tag="ot",
                                                 name=f"ot0_{th}_{tt}")
                                    nc.vector.tensor_add(
                                        ot[:], pss[tt][:],
                                        bias[:, 0:OT])
                                    nc.sync.dma_start(
                                        out[t0+tt*TT:t0+(tt+1)*TT, 0:OT],
                                        ot[:])
                        continue
                    wts = []
                    for k in range(NKB):
                        wt = wp.tile([KT, OT], bf16, tag="w")
                        nc.sync.dma_start(
                            wt[:], wT[k*KT:(k+1)*KT, o*OT:(o+1)*OT])
                        wts.append(wt)
                    w8t = wp8.tile([128, KFB, OT], f8, tag="w8")
                    nc.sync.dma_start(w8t[:], w8d[o])
                    for tt in range(NT):
                        ps = pp.tile([TT, OT], f32, tag="ps")
                        for k in range(NKB):
                            nc.tensor.matmul(
                                ps[:], xts[k][:, tt*TT:(tt+1)*TT], wts[k][:],
                                start=(k == 0), stop=False)
                        for kp in range(NKP):
                            nc.tensor.matmul(
                                ps[:],
                                x8t[:, 2*kp:2*kp+2, tt*TT:(tt+1)*TT],
                                w8t[:, 2*kp:2*kp+2, :],
                                start=False, stop=(kp == NKP - 1),
                                perf_mode=DR)
                        ot = op.tile([TT, OT], f32, tag="ot")
                        nc.vector.tensor_add(
                            ot[:], ps[:], bias[:, o*OT:(o+1)*OT])
                        nc.sync.dma_start(
                            out[t0+tt*TT:t0+(tt+1)*TT, o*OT:(o+1)*OT], ot[:])
    _split_multi_waits(nc)
    return nc


_NC_CACHE = []


def _get_nc():
    if not _NC_CACHE:
        _NC_CACHE.append(build_nc())
    return _NC_CACHE[0]


def make_in_maps(x, W0, b, lokr_w1, lokr_w2_a, lokr_w2_b):
    scale = (ALPHA / LORA_DIM) * MULTIPLIER
    w2 = lokr_w2_a.astype(np.float32) @ lokr_w2_b.astype(np.float32)
    w_eff = (W0.astype(np.float32) + scale * np.kron(
        lokr_w1.astype(np.float32), w2)) * WSCALE
    wTf = np.ascontiguousarray(w_eff.T)          # [IN, OUT], pre-scaled
    wT_bf = wTf[:KB].astype(ml_dtypes.bfloat16)
    w8 = wTf[KB:].astype(ml_dtypes.float8_e4m3fn)        # [KF, OUT]
    # [KFB,128,NO,OT] -> [NO, 128, KFB, OT]
    w8p = np.ascontiguousarray(
        w8.reshape(KFB, 128, NO, OT).transpose(2, 1, 0, 3))
    bias_rep = np.ascontiguousarray(np.broadcast_to(
        (b.astype(np.float32) * WSCALE)[None, :], (128, OUT)))
    xs = x.astype(np.float32).reshape(B * S, IN)
    in_maps = []
    for c in range(N_CORES):
        shard = xs[c*T_CORE:(c+1)*T_CORE]
        xT_f = np.ascontiguousarray(shard.T)     # [IN, T_CORE]
        xT_bf = xT_f[:KB].astype(ml_dtypes.bfloat16)
        x8 = xT_f[KB:].astype(ml_dtypes.float8_e4m3fn)   # [KF, T_CORE]
        # [KFB,128,2,T_HALF] -> [2, 128, KFB, T_HALF]
        x8p = np.ascontiguousarray(
            x8.reshape(KFB, 128, 2, T_HALF).transpose(2, 1, 0, 3))
        in_maps.append({"xT": xT_bf, "x8": x8p, "wT": wT_bf, "w8": w8p,
                        "bias": bias_rep})
    return in_maps


def run_spmd(in_maps, trace=False, **kw):
    nc = _get_nc()
    return bass_utils.run_bass_kernel_spmd(
        nc, in_maps, core_ids=list(range(N_CORES)), trace=trace, **kw)


def kernel(x, W0, b, lokr_w1, lokr_w2_a, lokr_w2_b):
    in_maps = make_in_maps(x, W0, b, lokr_w1, lokr_w2_a, lokr_w2_b)
    res = run_spmd(in_maps, trace=False)
    out = np.concatenate(
        [res.results[c]["out"] for c in range(N_CORES)], axis=0)
    return (out.reshape(B, S, OUT) * np.float32(1.0 / WSCALE)
            ).astype(np.float32)


# revision 5
# speedup vs baseline: 1.2168x; 1.2168x over previous
"""LoKr linear forward on 8 TRN2 NeuronCores — hybrid bf16/fp8 matmul.

out = x @ (W0 + (alpha/lora_dim) * kron(w1, w2_a @ w2_b)).T + b

Same data-parallel token sharding as the baseline, but the K=4096
contraction is split: the first KB dims run in bf16 (exact to ~1e-3),
the remaining KF dims run in fp8-e4m3 DoubleRow mode (2x TensorE rate,
256-deep contraction per instruction). Measured rel err ~1.8e-2 < 2e-2.

Weights (both parts) and bias are pre-scaled by 32 on host so the fp8
weight values clear the e4m3 subnormal range; the kernel output is
32*(x@W+b) and the host multiplies by 1/32 (exact) after gather.
"""
import sys, types

sys.path.insert(0, '/opt/trn_rl_repo')

import numpy as np
import ml_dtypes
import concourse.bass as bass
import concourse.mybir as mybir
import concourse.tile as tile
import concourse.bass_utils as bass_utils

ALPHA = 1.0
LORA_DIM = 4
MULTIPLIER = 1.0

N_CORES = 8
B, S, IN, OUT = 4, 4096, 4096, 4096
T_CORE = B * S // N_CORES          # 2048 tokens per core
T_HALF = T_CORE // 2               # 1024
KT = 128                           # bf16 contraction tile
TT = 128                           # token tile (psum partitions)
OT = 512                           # out-feature tile (psum free dim)
KB = 2560                          # bf16 contraction span
KF = IN - KB                       # fp8 contraction span (1280)
NKB = KB // KT                     # 22 bf16 k-tiles
NKP = KF // 256                    # 5 fp8 DoubleRow k-pair-tiles
KFB = KF // 128                    # 10 fp8 k-blocks
NO = OUT // OT                     # 8
NT = T_HALF // TT                  # 8
WSCALE = 32.0


def _split_multi_waits(nc):
    """This walrus build encodes at most ONE semaphore wait per ISA
    instruction; hoist extra waits onto single-wait NOPs inserted before."""
    ctr = 0
    for f in nc.m.functions:
        for blk in f.blocks:
            out = []
            changed = False
            for i in blk.instructions:
                si = i.sync_info
                if si is not None and si.on_wait and len(si.on_wait) > 1:
                    waits = list(si.on_wait)
                    for w in waits[:-1]:
                        ctr += 1
                        out.append(mybir.InstNoOp(
                            name=f"I-wsplit-{ctr}",
                            engine=i.engine, ins=[], outs=[],
                            sync_info=mybir.SyncInfo(on_wait=[w], on_update=[]),
                        ))
                    i.sync_info = mybir.SyncInfo(
                        on_wait=[waits[-1]], on_update=list(si.on_update))
                    changed = True
                out.append(i)
            if changed:
                blk.instructions = out


def build_nc():
    nc = bass.Bass(trn_type="TRN2")
    bf16 = mybir.dt.bfloat16
    f8 = mybir.dt.float8e4
    f32 = mybir.dt.float32
    DR = mybir.MatmulPerfMode.DoubleRow
    xT = nc.dram_tensor("xT", [KB, T_CORE], bf16, kind="ExternalInput")
    # x8 packed per half: [th][128 p][KFB][1024 t], value = x.T[KB+kb*128+p, t]
    x8d = nc.dram_tensor("x8", [2, 128, KFB, T_HALF], f8, kind="ExternalInput")
    wT = nc.dram_tensor("wT", [KB, OUT], bf16, kind="ExternalInput")
    # w8 packed per o-block: [ob][128 p][KFB][512 o]
    w8d = nc.dram_tensor("w8", [NO, 128, KFB, OT], f8, kind="ExternalInput")
    bias_d = nc.dram_tensor("bias", [128, OUT], f32, kind="ExternalInput")
    out = nc.dram_tensor("out", [T_CORE, OUT], f32, kind="ExternalOutput")

    with tile.TileContext(nc) as tc:
        with (
            tc.tile_pool(name="const", bufs=1) as constp,
            tc.tile_pool(name="warm", bufs=1) as warmp,
            tc.tile_pool(name="xp", bufs=NKB + 4) as xp,
            tc.tile_pool(name="xp8", bufs=2) as xp8,
            tc.tile_pool(name="w0p", bufs=1) as w0p,       # o=0 W, resident
            tc.tile_pool(name="wp", bufs=2 * NKB) as wp,
            tc.tile_pool(name="wp8", bufs=2) as wp8,
            tc.tile_pool(name="op", bufs=8) as op,
            tc.tile_pool(name="ps", bufs=8, space="PSUM") as pp,
        ):
            # PE warm-up: keep TensorE busy while the first DMAs land so the
            # HAM clock-gate opens before real matmuls start. (wz is read
            # uninitialized on purpose; the result is never consumed.)
            wz = warmp.tile([KT, OT], bf16)
            nc.sync.dma_start(wz[:], wT[0:KT, 0:OT])
            wps = pp.tile([TT, OT], f32, tag="ps")
            for _ in range(96):
                nc.tensor.matmul(wps[:, :TT], wz[:, :TT], wz[:, :TT],
                                 start=True, stop=True)

            bias = constp.tile([128, OUT], f32)

            # o=0 weights are th-invariant: load once, keep resident.
            # NOTE: this pre-compute DMA wave (before the first real matmul)
            # is deliberate: the ~15-20us PE-idle window after the warm-up
            # burst is what lets the HAM clock-gate step the PE clock to
            # 2.4 GHz. A "better-overlapped" schedule that keeps the PE
            # continuously busy from t=0 leaves the clock stuck ~20% lower
            # for the whole kernel (measured 890us vs 763us).
            w0ts = []
            for k in range(NKB):
                wt = w0p.tile([KT, OT], bf16, name=f"w0_{k}")
                nc.sync.dma_start(wt[:], wT[k*KT:(k+1)*KT, 0:OT])
                w0ts.append(wt)
            w08 = w0p.tile([128, KFB, OT], f8, name="w0_f8")
            nc.sync.dma_start(w08[:], w8d[0])

            for th in range(2):
                t0 = th * T_HALF
                # fp8 x half-shard first (one big DMA), then bf16 x tiles.
                x8t = xp8.tile([128, KFB, T_HALF], f8, tag="x8")
                nc.sync.dma_start(x8t[:], x8d[th])
                xts = []
                for k in range(NKB):
                    xt = xp.tile([KT, T_HALF], bf16, tag="x",
                                 name=f"x_{th}_{k}")
                    nc.sync.dma_start(
                        xt[:], xT[k*KT:(k+1)*KT, t0:t0+T_HALF])
                    xts.append(xt)
                if th == 0:
                    nc.sync.dma_start(bias[:], bias_d[:])
                for o in range(NO):
                    if o == 0:
                        # k-outer / t-inner across all 8 PSUM banks: the PE
                        # consumes each freshly-DMA'd tile for all 8 token
                        # tiles at once, tracking the DMA wavefront.
                        pss = [pp.tile([TT, OT], f32, tag="ps",
                                        name=f"pss_{th}_{i}")
                               for i in range(NT)]
                        for k in range(NKB):
                            for tt in range(NT):
                                nc.tensor.matmul(
                                    pss[tt][:],
                                    xts[k][:, tt*TT:(tt+1)*TT], w0ts[k][:],
                                    start=(k == 0), stop=False)
                        for kp in range(NKP):
                            for tt in range(NT):
                                nc.tensor.matmul(
                                    pss[tt][:],
                                    x8t[:, 2*kp:2*kp+2, tt*TT:(tt+1)*TT],
                                    w08[:, 2*kp:2*kp+2, :],
                                    start=False, stop=(kp == NKP - 1),
                                    perf_mode=DR)
                                if kp == NKP - 1:
                                    ot = op.tile([TT, OT], f32, tag="ot",
                                                 name=f"ot0_{th}_{tt}")
                                    nc.vector.tensor_add(
                                        ot[:], pss[tt][:],
                                        bias[:, 0:OT])
                                    nc.sync.dma_start(
                                        out[t0+tt*TT:t0+(tt+1)*TT, 0:OT],
                                        ot[:])
                        continue
                    wts = []
                    for k in range(NKB):
                        wt = wp.tile([KT, OT], bf16, tag="w")
                        nc.sync.dma_start(
                            wt[:], wT[k*KT:(k+1)*KT, o*OT:(o+1)*OT])
                        wts.append(wt)
                    w8t = wp8.tile([128, KFB, OT], f8, tag="w8")
                    nc.sync.dma_start(w8t[:], w8d[o])
                    for tt in range(NT):
                        ps = pp.tile([TT, OT], f32, tag="ps")
                        for k in range(NKB):
                            nc.tensor.matmul(
                                ps[:], xts[k][:, tt*TT:(tt+1)*TT], wts[k][:],
                                start=(k == 0), stop=False)
                        for kp in range(NKP):
                            nc.tensor.matmul(
                                ps[:],
                                x8t[:, 2*kp:2*kp+2, tt*TT:(tt+1)*TT],
                                w8t[:, 2*kp:2*kp+2, :],
                                start=False, stop=(kp == NKP - 1),
                                perf_mode=DR)
                        ot = op.tile([TT, OT], f32, tag="ot")
                        nc.vector.tensor_add(
                            ot[:], ps[:], bias[:, o*OT:(o+1)*OT])
                        nc.sync.dma_start(
                            out[t0+tt*TT:t0+(tt+1)*TT, o*OT:(o+1)*OT], ot[:])
    _split_multi_waits(nc)
    return nc


_NC_CACHE = []


def _get_nc():
    if not _NC_CACHE:
        _NC_CACHE.append(build_nc())
    return _NC_CACHE[0]


def make_in_maps(x, W0, b, lokr_w1, lokr_w2_a, lokr_w2_b):
    scale = (ALPHA / LORA_DIM) * MULTIPLIER
    w2 = lokr_w2_a.astype(np.float32) @ lokr_w2_b.astype(np.float32)
    w_eff = (W0.astype(np.float32) + scale * np.kron(
        lokr_w1.astype(np.float32), w2)) * WSCALE
    wTf = np.ascontiguousarray(w_eff.T)          # [IN, OUT], pre-scaled
    wT_bf = wTf[:KB].astype(ml_dtypes.bfloat16)
    w8 = wTf[KB:].astype(ml_dtypes.float8_e4m3fn)        # [KF, OUT]
    # [KFB,128,NO,OT] -> [NO, 128, KFB, OT]
    w8p = np.ascontiguousarray(
        w8.reshape(KFB, 128, NO, OT).transpose(2, 1, 0, 3))
    bias_rep = np.ascontiguousarray(np.broadcast_to(
        (b.astype(np.float32) * WSCALE)[None, :], (128, OUT)))
    xs = x.astype(np.float32).reshape(B * S, IN)
    in_maps = []
    for c in range(N_CORES):
        shard = xs[c*T_CORE:(c+1)*T_CORE]
        xT_f = np.ascontiguousarray(shard.T)     # [IN, T_CORE]
        xT_bf = xT_f[:KB].astype(ml_dtypes.bfloat16)
        x8 = xT_f[KB:].astype(ml_dtypes.float8_e4m3fn)   # [KF, T_CORE]
        # [KFB,128,2,T_HALF] -> [2, 128, KFB, T_HALF]
        x8p = np.ascontiguousarray(
            x8.reshape(KFB, 128, 2, T_HALF).transpose(2, 1, 0, 3))
        in_maps.append({"xT": xT_bf, "x8": x8p, "wT": wT_bf, "w8": w8p,
                        "bias": bias_rep})
    return in_maps


def run_spmd(in_maps, trace=False, **kw):
    nc = _get_nc()
    return bass_utils.run_bass_kernel_spmd(
        nc, in_maps, core_ids=list(range(N_CORES)), trace=trace, **kw)


def kernel(x, W0, b, lokr_w1, lokr_w2_a, lokr_w2_b):
    in_maps = make_in_maps(x, W0, b, lokr_w1, lokr_w2_a, lokr_w2_b)
    res = run_spmd(in_maps, trace=False)
    out = np.concatenate(
        [res.results[c]["out"] for c in range(N_CORES)], axis=0)
    return (out.reshape(B, S, OUT) * np.float32(1.0 / WSCALE)
            ).astype(np.float32)


# revision 6
# speedup vs baseline: 1.2207x; 1.0032x over previous
"""LoKr linear forward on 8 TRN2 NeuronCores — hybrid bf16/fp8 matmul.

out = x @ (W0 + (alpha/lora_dim) * kron(w1, w2_a @ w2_b)).T + b

Same data-parallel token sharding as the baseline, but the K=4096
contraction is split: the first KB dims run in bf16 (exact to ~1e-3),
the remaining KF dims run in fp8-e4m3 DoubleRow mode (2x TensorE rate,
256-deep contraction per instruction). Measured rel err 1.9514e-2 < 2e-2,
bit-exact with the ml_dtypes numpy simulation of the same quantization.

Weights (both parts) and bias are pre-scaled by 32 on host so the fp8
weight values clear the e4m3 subnormal range; the kernel output is
32*(x@W+b) and the host multiplies by 1/32 (exact) after gather.
"""
import sys, types

sys.path.insert(0, '/opt/trn_rl_repo')

import numpy as np
import ml_dtypes
import concourse.bass as bass
import concourse.mybir as mybir
import concourse.tile as tile
import concourse.bass_utils as bass_utils

ALPHA = 1.0
LORA_DIM = 4
MULTIPLIER = 1.0

N_CORES = 8
B, S, IN, OUT = 4, 4096, 4096, 4096
T_CORE = B * S // N_CORES          # 2048 tokens per core
T_HALF = T_CORE // 2               # 1024
KT = 128                           # bf16 contraction tile
TT = 128                           # token tile (psum partitions)
OT = 512                           # out-feature tile (psum free dim)
KB = 2560                          # bf16 contraction span
KF = IN - KB                       # fp8 contraction span (1536)
NKB = KB // KT                     # 20 bf16 k-tiles
NKP = KF // 256                    # 6 fp8 DoubleRow k-pair-tiles
KFB = KF // 128                    # 12 fp8 k-blocks
NO = OUT // OT                     # 8
NT = T_HALF // TT                  # 8
WSCALE = 32.0


def _split_multi_waits(nc):
    """This walrus build encodes at most ONE semaphore wait per ISA
    instruction; hoist extra waits onto single-wait NOPs inserted before."""
    ctr = 0
    for f in nc.m.functions:
        for blk in f.blocks:
            out = []
            changed = False
            for i in blk.instructions:
                si = i.sync_info
                if si is not None and si.on_wait and len(si.on_wait) > 1:
                    waits = list(si.on_wait)
                    for w in waits[:-1]:
                        ctr += 1
                        out.append(mybir.InstNoOp(
                            name=f"I-wsplit-{ctr}",
                            engine=i.engine, ins=[], outs=[],
                            sync_info=mybir.SyncInfo(on_wait=[w], on_update=[]),
                        ))
                    i.sync_info = mybir.SyncInfo(
                        on_wait=[waits[-1]], on_update=list(si.on_update))
                    changed = True
                out.append(i)
            if changed:
                blk.instructions = out


def build_nc():
    nc = bass.Bass(trn_type="TRN2")
    bf16 = mybir.dt.bfloat16
    f8 = mybir.dt.float8e4
    f32 = mybir.dt.float32
    DR = mybir.MatmulPerfMode.DoubleRow
    xT = nc.dram_tensor("xT", [KB, T_CORE], bf16, kind="ExternalInput")
    # x8 packed per half: [th][128 p][KFB][1024 t], value = x.T[KB+kb*128+p, t]
    x8d = nc.dram_tensor("x8", [2, 128, KFB, T_HALF], f8, kind="ExternalInput")
    wT = nc.dram_tensor("wT", [KB, OUT], bf16, kind="ExternalInput")
    # w8 packed per o-block: [ob][128 p][KFB][512 o]
    w8d = nc.dram_tensor("w8", [NO, 128, KFB, OT], f8, kind="ExternalInput")
    bias_d = nc.dram_tensor("bias", [128, OUT], f32, kind="ExternalInput")
    out = nc.dram_tensor("out", [T_CORE, OUT], f32, kind="ExternalOutput")

    with tile.TileContext(nc) as tc:
        with (
            tc.tile_pool(name="const", bufs=1) as constp,
            tc.tile_pool(name="warm", bufs=1) as warmp,
            tc.tile_pool(name="xp", bufs=NKB + 4) as xp,
            tc.tile_pool(name="xp8", bufs=2) as xp8,
            tc.tile_pool(name="w0p", bufs=1) as w0p,       # o=0 W, resident
            tc.tile_pool(name="wp", bufs=2 * NKB) as wp,
            tc.tile_pool(name="wp8", bufs=2) as wp8,
            tc.tile_pool(name="op", bufs=8) as op,
            tc.tile_pool(name="ps", bufs=8, space="PSUM") as pp,
        ):
            # PE warm-up: keep TensorE busy while the first DMAs land so the
            # HAM clock-gate opens before real matmuls start. (wz is read
            # uninitialized on purpose; the result is never consumed.)
            wz = warmp.tile([KT, OT], bf16)
            nc.sync.dma_start(wz[:], wT[0:KT, 0:OT])
            wps = pp.tile([TT, OT], f32, tag="ps")
            for _ in range(96):
                nc.tensor.matmul(wps[:, :TT], wz[:, :TT], wz[:, :TT],
                                 start=True, stop=True)

            bias = constp.tile([128, OUT], f32)

            # o=0 weights are th-invariant: load once, keep resident.
            # NOTE: this pre-compute DMA wave (before the first real matmul)
            # is deliberate: the ~15-20us PE-idle window after the warm-up
            # burst is what lets the HAM clock-gate step the PE clock to
            # 2.4 GHz. A "better-overlapped" schedule that keeps the PE
            # continuously busy from t=0 leaves the clock stuck ~20% lower
            # for the whole kernel (measured 890us vs 763us).
            w0ts = []
            for k in range(NKB):
                wt = w0p.tile([KT, OT], bf16, name=f"w0_{k}")
                nc.sync.dma_start(wt[:], wT[k*KT:(k+1)*KT, 0:OT])
                w0ts.append(wt)
            w08 = w0p.tile([128, KFB, OT], f8, name="w0_f8")
            nc.sync.dma_start(w08[:], w8d[0])

            for th in range(2):
                t0 = th * T_HALF
                # fp8 x half-shard first (one big DMA), then bf16 x tiles.
                x8t = xp8.tile([128, KFB, T_HALF], f8, tag="x8")
                nc.sync.dma_start(x8t[:], x8d[th])
                xts = []
                for k in range(NKB):
                    xt = xp.tile([KT, T_HALF], bf16, tag="x",
                                 name=f"x_{th}_{k}")
                    nc.sync.dma_start(
                        xt[:], xT[k*KT:(k+1)*KT, t0:t0+T_HALF])
                    xts.append(xt)
                if th == 0:
                    nc.sync.dma_start(bias[:], bias_d[:])
                for o in range(NO):
                    if o == 0:
                        # k-outer / t-inner across all 8 PSUM banks: the PE
                        # consumes each freshly-DMA'd tile for all 8 token
                        # tiles at once, tracking the DMA wavefront.
                        pss = [pp.tile([TT, OT], f32, tag="ps",
                                        name=f"pss_{th}_{i}")
                               for i in range(NT)]
                        for k in range(NKB):
                            for tt in range(NT):
                                nc.tensor.matmul(
                                    pss[tt][:],
                                    xts[k][:, tt*TT:(tt+1)*TT], w0ts[k][:],
                                    start=(k == 0), stop=False)
                        for kp in range(NKP):
                            for tt in range(NT):
                                nc.tensor.matmul(
                                    pss[tt][:],
                                    x8t[:, 2*kp:2*kp+2, tt*TT:(tt+1)*TT],
                                    w08[:, 2*kp:2*kp+2, :],
                                    start=False, stop=(kp == NKP - 1),
                                    perf_mode=DR)
                                if kp == NKP - 1:
                                    ot = op.tile([TT, OT], f32, tag="ot",
                                                 name=f"ot0_{th}_{tt}")
                                    nc.vector.tensor_add(
                                        ot[:], pss[tt][:],
                                        bias[:, 0:OT])
                                    nc.sync.dma_start(
                                        out[t0+tt*TT:t0+(tt+1)*TT, 0:OT],
                                        ot[:])
                        continue
                    wts = []
                    for k in range(NKB):
                        wt = wp.tile([KT, OT], bf16, tag="w")
                        nc.sync.dma_start(
                            wt[:], wT[k*KT:(k+1)*KT, o*OT:(o+1)*OT])
                        wts.append(wt)
                    w8t = wp8.tile([128, KFB, OT], f8, tag="w8")
                    nc.sync.dma_start(w8t[:], w8d[o])
                    for tt in range(NT):
                        ps = pp.tile([TT, OT], f32, tag="ps")
                        for k in range(NKB):
                            nc.tensor.matmul(
                                ps[:], xts[k][:, tt*TT:(tt+1)*TT], wts[k][:],
                                start=(k == 0), stop=False)
                        for kp in range(NKP):
                            nc.tensor.matmul(
                                ps[:],
                                x8t[:, 2*kp:2*kp+2, tt*TT:(tt+1)*TT],
                                w8t[:, 2*kp:2*kp+2, :],
                                start=False, stop=(kp == NKP - 1),
                                perf_mode=DR)
                        ot = op.tile([TT, OT], f32, tag="ot")
                        nc.vector.tensor_add(
                            ot[:], ps[:], bias[:, o*OT:(o+1)*OT])
                        nc.sync.dma_start(
                            out[t0+tt*TT:t0+(tt+1)*TT, o*OT:(o+1)*OT], ot[:])
    _split_multi_waits(nc)
    return nc


_NC_CACHE = []


def _get_nc():
    if not _NC_CACHE:
        _NC_CACHE.append(build_nc())
    return _NC_CACHE[0]


def make_in_maps(x, W0, b, lokr_w1, lokr_w2_a, lokr_w2_b):
    scale = (ALPHA / LORA_DIM) * MULTIPLIER
    w2 = lokr_w2_a.astype(np.float32) @ lokr_w2_b.astype(np.float32)
    w_eff = (W0.astype(np.float32) + scale * np.kron(
        lokr_w1.astype(np.float32), w2)) * WSCALE
    wTf = np.ascontiguousarray(w_eff.T)          # [IN, OUT], pre-scaled
    wT_bf = wTf[:KB].astype(ml_dtypes.bfloat16)
    w8 = wTf[KB:].astype(ml_dtypes.float8_e4m3fn)        # [KF, OUT]
    # [KFB,128,NO,OT] -> [NO, 128, KFB, OT]
    w8p = np.ascontiguousarray(
        w8.reshape(KFB, 128, NO, OT).transpose(2, 1, 0, 3))
    bias_rep = np.ascontiguousarray(np.broadcast_to(
        (b.astype(np.float32) * WSCALE)[None, :], (128, OUT)))
    xs = x.astype(np.float32).reshape(B * S, IN)
    in_maps = []
    for c in range(N_CORES):
        shard = xs[c*T_CORE:(c+1)*T_CORE]
        xT_f = np.ascontiguousarray(shard.T)     # [IN, T_CORE]
        xT_bf = xT_f[:KB].astype(ml_dtypes.bfloat16)
        x8 = xT_f[KB:].astype(ml_dtypes.float8_e4m3fn)   # [KF, T_CORE]
        # [KFB,128,2,T_HALF] -> [2, 128, KFB, T_HALF]
        x8p = np.ascontiguousarray(
            x8.reshape(KFB, 128, 2, T_HALF).transpose(2, 1, 0, 3))
        in_maps.append({"xT": xT_bf, "x8": x8p, "wT": wT_bf, "w8": w8p,
                        "bias": bias_rep})
    return in_maps


def run_spmd(in_maps, trace=False, **kw):
    nc = _get_nc()
    return bass_utils.run_bass_kernel_spmd(
        nc, in_maps, core_ids=list(range(N_CORES)), trace=trace, **kw)


def kernel(x, W0, b, lokr_w1, lokr_w2_a, lokr_w2_b):
    in_maps = make_in_maps(x, W0, b, lokr_w1, lokr_w2_a, lokr_w2_b)
    res = run_spmd(in_maps, trace=False)
    out = np.concatenate(
        [res.results[c]["out"] for c in range(N_CORES)], axis=0)
    return (out.reshape(B, S, OUT) * np.float32(1.0 / WSCALE)
            ).astype(np.float32)
